# revision 5
# baseline (speedup 1.0000x reference)
"""Chamfer distance kernel for Trainium2 (Bass/Tile), SPMD over 8 NeuronCores.

Problem: input1 [8, 4096, 64], input2 [8, 4096, 64] (fp32).
    D[b,n,m] = ||x_bn - y_bm||_2
    loss = mean_b( mean_m(min_n D) + mean_n(min_m D) )

Sharding: data-parallel over batch B=8 -> one batch element per core.

Per-core algorithm (flash-style, the [N, M] matrix never hits HBM):
  - Build augmented K-major operands so ONE matmul produces the full squared
    distance tile directly in PSUM:
        lhsT = [ -2*X^T ; x2 ; 1 ]   (66 x 128 per n-tile)
        rhs  = [  Y^T   ; 1  ; y2 ]  (66 x 512 per m-tile)
        psum[n, m] = x2[n] + y2[m] - 2*<x_n, y_m>   = d^2
  - ScalarE copies each psum tile to SBUF as fp16 (min-selection in fp16 is
    exact-to-selection; verified loss rel-err ~4e-7).
  - VectorE per tile:
      * fused tensor_tensor_reduce: rowacc[t] = min(rowacc[t], T); the last
        m-tile's accum_out yields rowmin (min over m) for free.
      * tensor_tensor min into colacc[j] accumulates min over n-tiles.
  - colacc partition-axis min via TensorE transpose + free-axis reduce_min.
  - Device returns raw squared minima [128, n_nt + 4*n_mt]; host does
    clamp/sqrt/mean (cheap: 8192 values per core).
"""

import sys

if "/opt/trn_rl_repo" not in sys.path:
    sys.path.insert(0, "/opt/trn_rl_repo")

import numpy as np

B = 8
N = 4096
M = 4096
K = 64
NT = 128          # n-tile (psum partition dim)
MT = 512          # m-tile (psum free dim, one PSUM bank fp32)
KA = K + 2        # augmented contraction
FP16_BIG = 60000.0
BIG = 3.0e38

_COMPILED = {}
LAST_RESULTS = None


def _build(n_rows, m_cols, num_cores):
    """Trace + compile the per-core bass program for [n_rows, K] x [m_cols, K]."""
    import concourse.bacc as bacc
    import concourse.mybir as mybir
    import concourse.tile as tile
    from concourse.masks import make_identity

    f32 = mybir.dt.float32
    f16 = mybir.dt.float16
    AX = mybir.AxisListType
    OP = mybir.AluOpType

    n_nt = n_rows // NT
    n_mt = m_cols // MT
    n_yt = m_cols // 128  # y transpose tiles

    nc = bacc.Bacc(
        "TRN2", target_bir_lowering=False, debug=False, num_devices=num_cores
    )
    xd = nc.dram_tensor("x", [n_rows, K], f32, kind="ExternalInput")
    yd = nc.dram_tensor("y", [m_cols, K], f32, kind="ExternalInput")
    outd = nc.dram_tensor("out", [128, n_nt + 4 * n_mt], f32, kind="ExternalOutput")

    with tile.TileContext(nc) as tc:
        with (
            tc.tile_pool(name="const", bufs=1) as cpool,
            tc.tile_pool(name="tsbp", bufs=4) as tsb_pool,
            tc.tile_pool(name="mpsum", bufs=4, space="PSUM") as ps_pool,
            tc.tile_pool(name="tpsum", bufs=2, space="PSUM") as tp_pool,
            tc.tile_pool(name="work", bufs=2) as wpool,
        ):
            # ---------------- Phase 0: load + build augmented operands -----
            xsb = cpool.tile([128, n_nt * K], f32, name="xsb")
            ysb = cpool.tile([128, n_yt * K], f32, name="ysb")
            nc.sync.dma_start(
                xsb.rearrange("p (t k) -> p t k", k=K),
                xd[:].rearrange("(t p) k -> p t k", p=128),
            )
            nc.sync.dma_start(
                ysb.rearrange("p (t k) -> p t k", k=K),
                yd[:].rearrange("(t p) k -> p t k", p=128),
            )

            ident32 = cpool.tile([128, 128], f32, name="ident32")
            make_identity(nc, ident32)
            ident16 = cpool.tile([128, 128], f16, name="ident16")
            make_identity(nc, ident16)

            # x2 / y2 per point: sum_k v^2, laid out [p, tile]
            # (ScalarE Square with accum_out sums over the free dim)
            x2t = cpool.tile([128, n_nt], f32, name="x2t")
            y2t = cpool.tile([128, n_yt], f32, name="y2t")
            FN = mybir.ActivationFunctionType
            for t in range(n_nt):
                sq = wpool.tile([128, K], f32, tag="sq", name="sq")
                nc.scalar.activation(
                    sq,
                    xsb[:, t * K : (t + 1) * K],
                    FN.Square,
                    accum_out=x2t[:, t : t + 1],
                )
            for t in range(n_yt):
                sq = wpool.tile([128, K], f32, tag="sq", name="sq")
                nc.scalar.activation(
                    sq,
                    ysb[:, t * K : (t + 1) * K],
                    FN.Square,
                    accum_out=y2t[:, t : t + 1],
                )

            # K-major operands via PE transpose
            xt_aug = cpool.tile([KA, n_rows], f32, name="xt_aug")
            yt_aug = cpool.tile([KA, m_cols], f32, name="yt_aug")
            for t in range(n_nt):
                tp = tp_pool.tile([128, 128], f32, tag="tp", name="tp")
                nc.tensor.transpose(
                    tp[:K, :], xsb[:, t * K : (t + 1) * K], ident32
                )
                nc.scalar.mul(xt_aug[0:K, t * 128 : (t + 1) * 128], tp[:K, :], -2.0)
            for t in range(n_yt):
                tp = tp_pool.tile([128, 128], f32, tag="tp", name="tp")
                nc.tensor.transpose(
                    tp[:K, :], ysb[:, t * K : (t + 1) * K], ident32
                )
                nc.scalar.copy(yt_aug[0:K, t * 128 : (t + 1) * 128], tp[:K, :])

            # x2 row (lhsT row 64) and y2 row (rhs row 65), ones rows
            # ones into both aug rows first (base partition 64 is legal),
            # then overwrite the data row via DMA (no partition-base limits).
            nc.vector.memset(xt_aug[K : K + 2, :], 1.0)
            nc.vector.memset(yt_aug[K : K + 2, :], 1.0)

            x2p = tp_pool.tile([128, 128], f32, tag="tp", name="x2p")
            nc.tensor.transpose(x2p[:n_nt, :], x2t, ident32)
            x2r = wpool.tile([n_nt, 128], f32, tag="x2r", name="x2r")
            nc.scalar.copy(x2r, x2p[:n_nt, :])
            nc.sync.dma_start(xt_aug[K : K + 1, :], x2r)

            y2p = tp_pool.tile([128, 128], f32, tag="tp", name="y2p")
            nc.tensor.transpose(y2p[:n_yt, :], y2t, ident32)
            y2r = wpool.tile([n_yt, 128], f32, tag="x2r", name="y2r")
            nc.scalar.copy(y2r, y2p[:n_yt, :])
            nc.sync.dma_start(yt_aug[K + 1 : K + 2, :], y2r)

            # ---------------- Phase 1: main flash loop ---------------------
            # t outer (lhsT loaded once per n-tile), j inner.
            rowmin2d = cpool.tile([128, n_nt], f32, name="rowmin2d")
            colmin2d = cpool.tile([128, 4 * n_mt], f32, name="colmin2d")
            colacc = [
                cpool.tile([128, MT], f16, tag=f"colacc{j}", name=f"colacc{j}")
                for j in range(n_mt)
            ]

            for t in range(n_nt):
                rowacc = wpool.tile([128, MT], f16, tag="rowacc", name="rowacc", bufs=3)
                for j in range(n_mt):
                    ps = ps_pool.tile([128, MT], f32, tag="ps", name="ps")
                    nc.tensor.matmul(
                        ps,
                        lhsT=xt_aug[:, t * 128 : (t + 1) * 128],
                        rhs=yt_aug[:, j * MT : (j + 1) * MT],
                        start=True,
                        stop=True,
                    )
                    tsb = tsb_pool.tile([128, MT], f16, tag="tsb", name="tsb")
                    nc.scalar.copy(tsb, ps)

                    if j == 0:
                        nc.vector.tensor_copy(rowacc, tsb)
                    else:
                        nc.vector.tensor_tensor(rowacc, tsb, rowacc, OP.min)
                    if t == 0:
                        nc.vector.tensor_copy(colacc[j], tsb)
                    else:
                        nc.vector.tensor_tensor(colacc[j], tsb, colacc[j], OP.min)

                # min over m for this n-tile (overlaps next t's matmuls)
                nc.vector.tensor_reduce(
                    rowmin2d[:, t : t + 1], rowacc, AX.X, OP.min
                )

            # partition-axis min of each colacc via PE transpose (tail)
            for j in range(n_mt):
                for s in range(4):
                    cp = tp_pool.tile([128, 128], f16, tag="cp", name="cp")
                    nc.tensor.transpose(
                        cp, colacc[j][:, s * 128 : (s + 1) * 128], ident16
                    )
                    nc.vector.tensor_reduce(
                        colmin2d[:, j * 4 + s : j * 4 + s + 1], cp, AX.X, OP.min
                    )

            # ---------------- Phase 2: writeback ---------------------------
            nc.sync.dma_start(outd[:, 0:n_nt], rowmin2d)
            nc.sync.dma_start(outd[:, n_nt : n_nt + 4 * n_mt], colmin2d)

    nc.compile()
    return nc


def _get(n_rows, m_cols, num_cores):
    key = (n_rows, m_cols, num_cores)
    if key not in _COMPILED:
        _COMPILED[key] = _build(n_rows, m_cols, num_cores)
    return _COMPILED[key]


def _run(x, y, n_rows, m_cols, num_cores, trace=False):
    """x, y: [num_cores, n_rows|m_cols, K] fp32. Returns per-core out arrays."""
    global LAST_RESULTS
    from concourse import bass_utils

    nc = _get(n_rows, m_cols, num_cores)
    in_maps = [
        {"x": np.ascontiguousarray(x[b]), "y": np.ascontiguousarray(y[b])}
        for b in range(num_cores)
    ]
    res = bass_utils.run_bass_kernel_spmd(
        nc, in_maps, core_ids=list(range(num_cores)), trace=trace
    )
    LAST_RESULTS = res
    return [r["out"] for r in res.results]


def _postprocess(outs, n_rows, m_cols):
    """Host-side unshard: clamp, sqrt, mean. outs: list of [128, n_nt+4*n_mt]."""
    n_nt = n_rows // NT
    total = 0.0
    for o in outs:
        rowmin = o[:, :n_nt].astype(np.float64)
        colmin = o[:, n_nt:].astype(np.float64)
        d1 = np.sqrt(np.maximum(rowmin, 0.0)).mean()  # min over m, mean over n
        d0 = np.sqrt(np.maximum(colmin, 0.0)).mean()  # min over n, mean over m
        total += d0 + d1
    return np.float32(total / len(outs))


def kernel(input1, input2):
    x = np.asarray(input1, dtype=np.float32)
    y = np.asarray(input2, dtype=np.float32)
    assert x.shape == (B, N, K) and y.shape == (B, M, K), (x.shape, y.shape)
    outs = _run(x, y, N, M, B)
    return _postprocess(outs, N, M)


# revision 7
# speedup vs baseline: 1.9491x; 1.9491x over previous
"""Chamfer distance kernel for Trainium2 (Bass/Tile), SPMD over 8 NeuronCores.

Problem: input1 [8, 4096, 64], input2 [8, 4096, 64] (fp32).
    D[b,n,m] = ||x_bn - y_bm||_2
    loss = mean_b( mean_m(min_n D) + mean_n(min_m D) )

Sharding: data-parallel over batch B=8 -> one batch element per core.

Per-core algorithm (flash-style, the [N, M] matrix never hits HBM):
  - Build augmented K-major fp16 operands so one matmul produces the full
    squared distance tile directly in PSUM (fp16 matmul streams at 1 cyc/col
    vs 4 for fp32; quantization impact on the final loss measured ~1e-6):
        lhsT = [ -2*X^T ; x2 ; 1 ]   (66 x 128 per n-tile)
        rhs  = [  Y^T   ; 1  ; y2 ]  (66 x 512 per m-tile)
        psum[n, m] = x2[n] + y2[m] - 2*<x_n, y_m>   = d^2
  - Two matmuls fill a 1024-wide 2-bank PSUM tile; ScalarE copies it to SBUF
    as fp16 (min-selection in fp16 is exact-to-selection).
  - VectorE per tile: running fp16 min into rowacc (per n-tile) and
    colacc[jj] (per m-superblock); both at the DVE 2x_1p rate.
  - colacc partition-axis min via TensorE transpose + free-axis reduce_min.
  - Device returns raw squared minima [128, n_nt + 4*n_mt]; host does
    clamp/sqrt/mean (cheap: 8192 values per core).
"""

import sys

if "/opt/trn_rl_repo" not in sys.path:
    sys.path.insert(0, "/opt/trn_rl_repo")

import numpy as np

B = 8
N = 4096
M = 4096
K = 64
NT = 128          # n-tile (psum partition dim)
MT = 512          # single-matmul moving free dim (one PSUM bank fp32)
JT = 1024         # m superblock (2 PSUM banks, one ACT/DVE op)
KA = K + 2        # augmented contraction

_COMPILED = {}
LAST_RESULTS = None


def _build(n_rows, m_cols, num_cores):
    """Trace + compile the per-core bass program for [n_rows, K] x [m_cols, K]."""
    import concourse.bacc as bacc
    import concourse.mybir as mybir
    import concourse.tile as tile
    from concourse.masks import make_identity

    f32 = mybir.dt.float32
    f16 = mybir.dt.float16
    AX = mybir.AxisListType
    OP = mybir.AluOpType
    FN = mybir.ActivationFunctionType

    n_nt = n_rows // NT
    n_jt = m_cols // JT
    n_yt = m_cols // 128  # y transpose tiles

    nc = bacc.Bacc(
        "TRN2", target_bir_lowering=False, debug=False, num_devices=num_cores
    )
    xd = nc.dram_tensor("x", [n_rows, K], f32, kind="ExternalInput")
    yd = nc.dram_tensor("y", [m_cols, K], f32, kind="ExternalInput")
    outd = nc.dram_tensor(
        "out", [128, n_nt + m_cols // 128], f32, kind="ExternalOutput"
    )

    with tile.TileContext(nc) as tc:
        with (
            tc.tile_pool(name="const", bufs=1) as cpool,
            tc.tile_pool(name="tsbp", bufs=4) as tsb_pool,
            tc.tile_pool(name="mpsum", bufs=3, space="PSUM") as ps_pool,
            tc.tile_pool(name="tpsum", bufs=2, space="PSUM") as tp_pool,
            tc.tile_pool(name="work", bufs=2) as wpool,
        ):
            # ---------------- Phase 0: load + build augmented operands -----
            xsb = cpool.tile([128, n_nt * K], f32, name="xsb")
            ysb = cpool.tile([128, n_yt * K], f32, name="ysb")
            nc.sync.dma_start(
                xsb.rearrange("p (t k) -> p t k", k=K),
                xd[:].rearrange("(t p) k -> p t k", p=128),
            )
            nc.sync.dma_start(
                ysb.rearrange("p (t k) -> p t k", k=K),
                yd[:].rearrange("(t p) k -> p t k", p=128),
            )

            ident32 = cpool.tile([128, 128], f32, name="ident32")
            make_identity(nc, ident32)
            ident16 = cpool.tile([128, 128], f16, name="ident16")
            make_identity(nc, ident16)

            # x2 / y2 per point: sum_k v^2, laid out [p, tile]
            # (ScalarE Square with accum_out sums over the free dim)
            x2t = cpool.tile([128, n_nt], f32, name="x2t")
            y2t = cpool.tile([128, n_yt], f32, name="y2t")
            for t in range(n_nt):
                sq = wpool.tile([128, K], f32, tag="sq", name="sq")
                nc.scalar.activation(
                    sq,
                    xsb[:, t * K : (t + 1) * K],
                    FN.Square,
                    accum_out=x2t[:, t : t + 1],
                )
            for t in range(n_yt):
                sq = wpool.tile([128, K], f32, tag="sq", name="sq")
                nc.scalar.activation(
                    sq,
                    ysb[:, t * K : (t + 1) * K],
                    FN.Square,
                    accum_out=y2t[:, t : t + 1],
                )

            # K-major fp16 operands via PE transpose (+ dtype cast on copy-out)
            xt_aug = cpool.tile([KA, n_rows], f16, name="xt_aug")
            yt_aug = cpool.tile([KA, m_cols], f16, name="yt_aug")
            for t in range(n_nt):
                tp = tp_pool.tile([128, 128], f32, tag="tp", name="tp")
                nc.tensor.transpose(
                    tp[:K, :], xsb[:, t * K : (t + 1) * K], ident32
                )
                nc.scalar.mul(xt_aug[0:K, t * 128 : (t + 1) * 128], tp[:K, :], -2.0)
            for t in range(n_yt):
                tp = tp_pool.tile([128, 128], f32, tag="tp", name="tp")
                nc.tensor.transpose(
                    tp[:K, :], ysb[:, t * K : (t + 1) * K], ident32
                )
                nc.scalar.copy(yt_aug[0:K, t * 128 : (t + 1) * 128], tp[:K, :])

            # ones into both aug rows first (base partition 64 is legal),
            # then overwrite the data row via DMA (no partition-base limits).
            nc.vector.memset(xt_aug[K : K + 2, :], 1.0)
            nc.vector.memset(yt_aug[K : K + 2, :], 1.0)

            x2p = tp_pool.tile([128, 128], f32, tag="tp", name="x2p")
            nc.tensor.transpose(x2p[:n_nt, :], x2t, ident32)
            x2r = wpool.tile([n_nt, 128], f16, tag="x2r", name="x2r")
            nc.scalar.copy(x2r, x2p[:n_nt, :])
            nc.sync.dma_start(xt_aug[K : K + 1, :], x2r)

            y2p = tp_pool.tile([128, 128], f32, tag="tp", name="y2p")
            nc.tensor.transpose(y2p[:n_yt, :], y2t, ident32)
            y2r = wpool.tile([n_yt, 128], f16, tag="x2r", name="y2r")
            nc.scalar.copy(y2r, y2p[:n_yt, :])
            nc.sync.dma_start(yt_aug[K + 1 : K + 2, :], y2r)

            # ---------------- Phase 1: main flash loop ---------------------
            # t outer, m-superblocks inner; 2 matmuls fill each 2-bank psum.
            rowmin2d = cpool.tile([128, n_nt], f32, name="rowmin2d")
            colmin2d = cpool.tile([128, m_cols // 128], f32, name="colmin2d")
            colacc = [
                cpool.tile([128, JT], f16, tag=f"colacc{j}", name=f"colacc{j}")
                for j in range(n_jt)
            ]

            for t in range(n_nt):
                rowacc = wpool.tile([128, JT], f16, tag="rowacc", name="rowacc", bufs=3)
                for jj in range(n_jt):
                    ps = ps_pool.tile([128, JT], f32, tag="ps", name="ps")
                    for h in range(JT // MT):
                        nc.tensor.matmul(
                            ps[:, h * MT : (h + 1) * MT],
                            lhsT=xt_aug[:, t * 128 : (t + 1) * 128],
                            rhs=yt_aug[
                                :, jj * JT + h * MT : jj * JT + (h + 1) * MT
                            ],
                            start=True,
                            stop=True,
                        )
                    tsb = tsb_pool.tile([128, JT], f16, tag="tsb", name="tsb")
                    nc.scalar.copy(tsb, ps)

                    if jj == 0:
                        nc.vector.tensor_copy(rowacc, tsb)
                    else:
                        nc.vector.tensor_tensor(rowacc, tsb, rowacc, OP.min)
                    if t == 0:
                        nc.vector.tensor_copy(colacc[jj], tsb)
                    else:
                        nc.vector.tensor_tensor(colacc[jj], tsb, colacc[jj], OP.min)

                # min over m for this n-tile (overlaps next t's matmuls)
                nc.vector.tensor_reduce(
                    rowmin2d[:, t : t + 1], rowacc, AX.X, OP.min
                )

            # partition-axis min of each colacc via PE transpose (tail)
            for jj in range(n_jt):
                for s in range(JT // 128):
                    cp = tp_pool.tile([128, 128], f16, tag="tp", name="cp")
                    nc.tensor.transpose(
                        cp, colacc[jj][:, s * 128 : (s + 1) * 128], ident16
                    )
                    nc.vector.tensor_reduce(
                        colmin2d[:, jj * 8 + s : jj * 8 + s + 1], cp, AX.X, OP.min
                    )

            # ---------------- Phase 2: writeback ---------------------------
            nc.sync.dma_start(outd[:, 0:n_nt], rowmin2d)
            nc.sync.dma_start(outd[:, n_nt:], colmin2d)

    nc.compile()
    return nc


def _get(n_rows, m_cols, num_cores):
    key = (n_rows, m_cols, num_cores)
    if key not in _COMPILED:
        _COMPILED[key] = _build(n_rows, m_cols, num_cores)
    return _COMPILED[key]


def _run(x, y, n_rows, m_cols, num_cores, trace=False):
    """x, y: [num_cores, n_rows|m_cols, K] fp32. Returns per-core out arrays."""
    global LAST_RESULTS
    from concourse import bass_utils

    nc = _get(n_rows, m_cols, num_cores)
    in_maps = [
        {"x": np.ascontiguousarray(x[b]), "y": np.ascontiguousarray(y[b])}
        for b in range(num_cores)
    ]
    res = bass_utils.run_bass_kernel_spmd(
        nc, in_maps, core_ids=list(range(num_cores)), trace=trace
    )
    LAST_RESULTS = res
    return [r["out"] for r in res.results]


def _postprocess(outs, n_rows, m_cols):
    """Host-side unshard: clamp, sqrt, mean. outs: list of [128, n_nt+m/128]."""
    n_nt = n_rows // NT
    total = 0.0
    for o in outs:
        rowmin = o[:, :n_nt].astype(np.float64)
        colmin = o[:, n_nt:].astype(np.float64)
        d1 = np.sqrt(np.maximum(rowmin, 0.0)).mean()  # min over m, mean over n
        d0 = np.sqrt(np.maximum(colmin, 0.0)).mean()  # min over n, mean over m
        total += d0 + d1
    return np.float32(total / len(outs))


def kernel(input1, input2):
    x = np.asarray(input1, dtype=np.float32)
    y = np.asarray(input2, dtype=np.float32)
    assert x.shape == (B, N, K) and y.shape == (B, M, K), (x.shape, y.shape)
    outs = _run(x, y, N, M, B)
    return _postprocess(outs, N, M)


# revision 12
# speedup vs baseline: 2.1673x; 1.1120x over previous
"""Chamfer distance kernel for Trainium2 (Bass/Tile), SPMD over 8 NeuronCores.

Problem: input1 [8, 4096, 64], input2 [8, 4096, 64] (fp32).
    D[b,n,m] = ||x_bn - y_bm||_2
    loss = mean_b( mean_m(min_n D) + mean_n(min_m D) )

Sharding: data-parallel over batch B=8 -> one batch element per core.

Per-core algorithm (flash-style, the [N, M] matrix never hits HBM):
  - Build augmented K-major fp16 operands so one matmul produces the full
    squared distance tile directly in PSUM (fp16 matmul streams at 1 cyc/col
    vs 4 for fp32; quantization impact on the final loss measured ~1e-6):
        lhsT = [ -2*X^T ; x2 ; 1 ]   (66 x 128 per n-tile)
        rhs  = [  Y^T   ; 1  ; y2 ]  (66 x 512 per m-tile)
        psum[n, m] = x2[n] + y2[m] - 2*<x_n, y_m>   = d^2
  - Four matmuls fill a 2048-wide 4-bank PSUM tile; ScalarE copies it to
    SBUF as fp16 (min-selection in fp16 is exact-to-selection). The first
    superblock copy lands directly in rowacc (saves a DVE copy).
  - VectorE: running fp16 min into rowacc (per n-tile, then fold+reduce to
    rowmin) and colacc[jj] (min over n-tiles) at the DVE 2x_1p rate.
  - colacc partition-axis min via TensorE transpose + free-axis reduce_min.
  - Device returns raw squared minima [128, n_nt + m/128]; host does
    clamp/sqrt/mean (cheap: 8192 values per core).
"""

import sys

if "/opt/trn_rl_repo" not in sys.path:
    sys.path.insert(0, "/opt/trn_rl_repo")

import numpy as np

B = 8
N = 4096
M = 4096
K = 64
NT = 128          # n-tile (psum partition dim)
MT = 512          # single-matmul moving free dim (one PSUM bank fp32)
KA = K + 2        # augmented contraction

_COMPILED = {}
LAST_RESULTS = None


def _build(n_rows, m_cols, num_cores):
    """Trace + compile the per-core bass program for [n_rows, K] x [m_cols, K]."""
    import concourse.bacc as bacc
    import concourse.mybir as mybir
    import concourse.tile as tile
    from concourse.masks import make_identity

    f32 = mybir.dt.float32
    f16 = mybir.dt.float16
    u32 = mybir.dt.uint32
    AX = mybir.AxisListType
    OP = mybir.AluOpType

    JT = min(2048, m_cols)      # m superblock (4 PSUM banks at 2048)
    n_nt = n_rows // NT
    n_jt = m_cols // JT
    n_yt = m_cols // 128        # y transpose tiles

    nc = bacc.Bacc(
        "TRN2", target_bir_lowering=False, debug=False, num_devices=num_cores
    )
    xd = nc.dram_tensor("x", [n_rows, K], f32, kind="ExternalInput")
    yd = nc.dram_tensor("y", [m_cols, K], f32, kind="ExternalInput")
    outd = nc.dram_tensor(
        "out", [128, n_nt + m_cols // 128], f32, kind="ExternalOutput"
    )

    with tile.TileContext(nc) as tc:
        with (
            tc.tile_pool(name="const", bufs=1) as cpool,
            tc.tile_pool(name="tsbp", bufs=4) as tsb_pool,
            tc.tile_pool(name="mpsum", bufs=2, space="PSUM") as ps_pool,
            tc.tile_pool(name="work", bufs=2) as wpool,
        ):
            # ---------------- Phase 0: load + build augmented operands -----
            xsb = cpool.tile([128, n_nt * K], f32, name="xsb")
            ysb = cpool.tile([128, n_yt * K], f32, name="ysb")
            nc.sync.dma_start(
                xsb.rearrange("p (t k) -> p t k", k=K),
                xd[:].rearrange("(t p) k -> p t k", p=128),
            )
            nc.sync.dma_start(
                ysb.rearrange("p (t k) -> p t k", k=K),
                yd[:].rearrange("(t p) k -> p t k", p=128),
            )

            ident32 = cpool.tile([128, 128], f32, name="ident32")
            make_identity(nc, ident32)
            ident16 = cpool.tile([128, 128], f16, name="ident16")
            make_identity(nc, ident16)

            # x2 / y2 per point: sum_k v^2, laid out [p, tile] (bulk DVE ops;
            # phase 0 is otherwise DVE-idle)
            x2t = cpool.tile([128, n_nt], f32, name="x2t")
            y2t = cpool.tile([128, n_yt], f32, name="y2t")
            xsq = wpool.tile([128, n_nt * K], f32, tag="xsq", name="xsq")
            nc.vector.tensor_tensor(xsq, xsb, xsb, OP.mult)
            nc.vector.tensor_reduce(
                x2t, xsq.rearrange("p (t k) -> p t k", k=K), AX.X, OP.add
            )
            ysq = wpool.tile([128, n_yt * K], f32, tag="xsq", name="ysq")
            nc.vector.tensor_tensor(ysq, ysb, ysb, OP.mult)
            nc.vector.tensor_reduce(
                y2t, ysq.rearrange("p (t k) -> p t k", k=K), AX.X, OP.add
            )

            # K-major fp16 operands via PE transpose (+ dtype cast on copy-out)
            xt_aug = cpool.tile([KA, n_rows], f16, name="xt_aug")
            yt_aug = cpool.tile([KA, m_cols], f16, name="yt_aug")
            for t in range(n_nt):
                tp = ps_pool.tile([128, JT], f32, tag="ps", name="tp")
                nc.tensor.transpose(
                    tp[:K, 0:128], xsb[:, t * K : (t + 1) * K], ident32
                )
                nc.scalar.mul(
                    xt_aug[0:K, t * 128 : (t + 1) * 128], tp[:K, 0:128], -2.0
                )
            for t in range(n_yt):
                tp = ps_pool.tile([128, JT], f32, tag="ps", name="tp")
                nc.tensor.transpose(
                    tp[:K, 0:128], ysb[:, t * K : (t + 1) * K], ident32
                )
                nc.scalar.copy(yt_aug[0:K, t * 128 : (t + 1) * 128], tp[:K, 0:128])

            # ones into both aug rows first (base partition 64 is legal),
            # then overwrite the data row via DMA (no partition-base limits).
            # bitcast to uint32 packs two fp16 1.0s per memset element.
            ONE2 = 0x3C003C00
            nc.vector.memset(xt_aug[K : K + 2, :].bitcast(u32), ONE2)
            nc.vector.memset(yt_aug[K : K + 2, :].bitcast(u32), ONE2)

            x2p = ps_pool.tile([128, JT], f32, tag="ps", name="x2p")
            nc.tensor.transpose(x2p[:n_nt, 0:128], x2t, ident32)
            x2r = wpool.tile([n_nt, 128], f16, tag="x2r", name="x2r")
            nc.scalar.copy(x2r, x2p[:n_nt, 0:128])
            nc.sync.dma_start(xt_aug[K : K + 1, :], x2r)

            y2p = ps_pool.tile([128, JT], f32, tag="ps", name="y2p")
            nc.tensor.transpose(y2p[:n_yt, 0:128], y2t, ident32)
            y2r = wpool.tile([n_yt, 128], f16, tag="x2r", name="y2r")
            nc.scalar.copy(y2r, y2p[:n_yt, 0:128])
            nc.sync.dma_start(yt_aug[K + 1 : K + 2, :], y2r)

            # ---------------- Phase 1: main flash loop ---------------------
            # t outer, m-superblocks inner; JT/MT matmuls fill each psum tile.
            rowmin2d = cpool.tile([128, n_nt], f32, name="rowmin2d")
            colmin2d = cpool.tile([128, m_cols // 128], f32, name="colmin2d")
            colacc = [
                cpool.tile([128, JT], f16, tag=f"colacc{j}", name=f"colacc{j}")
                for j in range(n_jt)
            ]

            for t in range(n_nt):
                rowacc = wpool.tile([128, JT], f16, tag="rowacc", name="rowacc", bufs=3)
                for jj in range(n_jt):
                    ps = ps_pool.tile([128, JT], f32, tag="ps", name="ps")
                    for h in range(JT // MT):
                        nc.tensor.matmul(
                            ps[:, h * MT : (h + 1) * MT],
                            lhsT=xt_aug[:, t * 128 : (t + 1) * 128],
                            rhs=yt_aug[
                                :, jj * JT + h * MT : jj * JT + (h + 1) * MT
                            ],
                            start=True,
                            stop=True,
                        )
                    if jj == 0:
                        # first superblock lands straight in rowacc
                        nc.scalar.copy(rowacc, ps)
                        src = rowacc
                    else:
                        tsb = tsb_pool.tile([128, JT], f16, tag="tsb", name="tsb")
                        nc.scalar.copy(tsb, ps)
                        nc.vector.tensor_tensor(rowacc, tsb, rowacc, OP.min)
                        src = tsb

                    if t == 0:
                        nc.vector.tensor_copy(colacc[jj], src)
                    else:
                        nc.vector.tensor_tensor(colacc[jj], src, colacc[jj], OP.min)

                # min over m for this n-tile (overlaps next t's matmuls):
                # fold halves twice with 2x TTs, then a 1x reduce on JT/4
                half = JT // 2
                nc.vector.tensor_tensor(
                    rowacc[:, 0:half], rowacc[:, 0:half], rowacc[:, half:JT], OP.min
                )
                quart = JT // 4
                nc.vector.tensor_tensor(
                    rowacc[:, 0:quart],
                    rowacc[:, 0:quart],
                    rowacc[:, quart : 2 * quart],
                    OP.min,
                )
                nc.vector.tensor_reduce(
                    rowmin2d[:, t : t + 1], rowacc[:, 0:quart], AX.X, OP.min
                )

            # partition-axis min of each colacc via PE transpose (tail)
            for jj in range(n_jt):
                for s in range(JT // 128):
                    cp = ps_pool.tile([128, JT], f16, tag="ps", name="cp")
                    nc.tensor.transpose(
                        cp[:, 0:128], colacc[jj][:, s * 128 : (s + 1) * 128], ident16
                    )
                    nc.vector.tensor_reduce(
                        colmin2d[:, jj * (JT // 128) + s : jj * (JT // 128) + s + 1],
                        cp[:, 0:128],
                        AX.X,
                        OP.min,
                    )

            # ---------------- Phase 2: writeback ---------------------------
            nc.sync.dma_start(outd[:, 0:n_nt], rowmin2d)
            nc.sync.dma_start(outd[:, n_nt:], colmin2d)

    nc.compile()
    return nc


def _get(n_rows, m_cols, num_cores):
    key = (n_rows, m_cols, num_cores)
    if key not in _COMPILED:
        _COMPILED[key] = _build(n_rows, m_cols, num_cores)
    return _COMPILED[key]


def _run(x, y, n_rows, m_cols, num_cores, trace=False):
    """x, y: [num_cores, n_rows|m_cols, K] fp32. Returns per-core out arrays."""
    global LAST_RESULTS
    from concourse import bass_utils

    nc = _get(n_rows, m_cols, num_cores)
    in_maps = [
        {"x": np.ascontiguousarray(x[b]), "y": np.ascontiguousarray(y[b])}
        for b in range(num_cores)
    ]
    res = bass_utils.run_bass_kernel_spmd(
        nc, in_maps, core_ids=list(range(num_cores)), trace=trace
    )
    LAST_RESULTS = res
    return [r["out"] for r in res.results]


def _postprocess(outs, n_rows, m_cols):
    """Host-side unshard: clamp, sqrt, mean. outs: list of [128, n_nt+m/128]."""
    n_nt = n_rows // NT
    total = 0.0
    for o in outs:
        rowmin = o[:, :n_nt].astype(np.float64)
        colmin = o[:, n_nt:].astype(np.float64)
        d1 = np.sqrt(np.maximum(rowmin, 0.0)).mean()  # min over m, mean over n
        d0 = np.sqrt(np.maximum(colmin, 0.0)).mean()  # min over n, mean over m
        total += d0 + d1
    return np.float32(total / len(outs))


def kernel(input1, input2):
    x = np.asarray(input1, dtype=np.float32)
    y = np.asarray(input2, dtype=np.float32)
    assert x.shape == (B, N, K) and y.shape == (B, M, K), (x.shape, y.shape)
    outs = _run(x, y, N, M, B)
    return _postprocess(outs, N, M)


# revision 15
# speedup vs baseline: 2.1772x; 1.0046x over previous
"""Chamfer distance kernel for Trainium2 (Bass/Tile), SPMD over 8 NeuronCores.

Problem: input1 [8, 4096, 64], input2 [8, 4096, 64] (fp32).
    D[b,n,m] = ||x_bn - y_bm||_2
    loss = mean_b( mean_m(min_n D) + mean_n(min_m D) )

Sharding: data-parallel over batch B=8 -> one batch element per core.

Per-core algorithm (flash-style, the [N, M] matrix never hits HBM):
  - Build augmented K-major fp16 operands so one matmul produces the full
    squared distance tile directly in PSUM (fp16 matmul streams at 1 cyc/col
    vs 4 for fp32; quantization impact on the final loss measured ~1e-6):
        lhsT = [ -2*X^T ; x2 ; 1 ]   (66 x 128 per n-tile)
        rhs  = [  Y^T   ; 1  ; y2 ]  (66 x 512 per m-tile)
        psum[n, m] = x2[n] + y2[m] - 2*<x_n, y_m>   = d^2
  - Four matmuls fill a 2048-wide 4-bank PSUM tile; ScalarE copies it to
    SBUF as fp16 (min-selection in fp16 is exact-to-selection). The first
    superblock copy lands directly in rowacc (saves a DVE copy).
  - VectorE: running fp16 min into rowacc (per n-tile, then fold+reduce to
    rowmin) and colacc[jj] (min over n-tiles) at the DVE 2x_1p rate.
  - colacc partition-axis min via TensorE transpose + free-axis reduce_min.
  - Device returns raw squared minima [128, n_nt + m/128]; host does
    clamp/sqrt/mean (cheap: 8192 values per core).
"""

import sys

if "/opt/trn_rl_repo" not in sys.path:
    sys.path.insert(0, "/opt/trn_rl_repo")

import numpy as np

B = 8
N = 4096
M = 4096
K = 64
NT = 128          # n-tile (psum partition dim)
MT = 512          # single-matmul moving free dim (one PSUM bank fp32)
KA = K + 2        # augmented contraction

_COMPILED = {}
LAST_RESULTS = None


def _build(n_rows, m_cols, num_cores):
    """Trace + compile the per-core bass program for [n_rows, K] x [m_cols, K]."""
    import concourse.bacc as bacc
    import concourse.mybir as mybir
    import concourse.tile as tile
    from concourse.masks import make_identity

    f32 = mybir.dt.float32
    f16 = mybir.dt.float16
    u32 = mybir.dt.uint32
    AX = mybir.AxisListType
    OP = mybir.AluOpType

    JT = min(2048, m_cols)      # m superblock (4 PSUM banks at 2048)
    n_nt = n_rows // NT
    n_jt = m_cols // JT
    n_yt = m_cols // 128        # y transpose tiles

    nc = bacc.Bacc(
        "TRN2", target_bir_lowering=False, debug=False, num_devices=num_cores
    )
    xd = nc.dram_tensor("x", [n_rows, K], f32, kind="ExternalInput")
    yd = nc.dram_tensor("y", [m_cols, K], f32, kind="ExternalInput")
    outd = nc.dram_tensor(
        "out", [128, n_nt + m_cols // 128], f32, kind="ExternalOutput"
    )

    with tile.TileContext(nc) as tc:
        with (
            tc.tile_pool(name="const", bufs=1) as cpool,
            tc.tile_pool(name="tsbp", bufs=4) as tsb_pool,
            tc.tile_pool(name="mpsum", bufs=2, space="PSUM") as ps_pool,
            tc.tile_pool(name="work", bufs=2) as wpool,
        ):
            # ---------------- Phase 0: load + build augmented operands -----
            xsb = cpool.tile([128, n_nt * K], f32, name="xsb")
            ysb = cpool.tile([128, n_yt * K], f32, name="ysb")
            nc.sync.dma_start(
                xsb.rearrange("p (t k) -> p t k", k=K),
                xd[:].rearrange("(t p) k -> p t k", p=128),
            )
            nc.sync.dma_start(
                ysb.rearrange("p (t k) -> p t k", k=K),
                yd[:].rearrange("(t p) k -> p t k", p=128),
            )

            ident32 = cpool.tile([128, 128], f32, name="ident32")
            make_identity(nc, ident32)
            ident16 = cpool.tile([128, 128], f16, name="ident16")
            make_identity(nc, ident16)

            # x2 / y2 per point: sum_k v^2, laid out [p, tile] (bulk DVE ops;
            # phase 0 is otherwise DVE-idle)
            x2t = cpool.tile([128, n_nt], f32, name="x2t")
            y2t = cpool.tile([128, n_yt], f32, name="y2t")
            xsq = wpool.tile([128, n_nt * K], f32, tag="xsq", name="xsq")
            nc.vector.tensor_tensor(xsq, xsb, xsb, OP.mult)
            nc.vector.tensor_reduce(
                x2t, xsq.rearrange("p (t k) -> p t k", k=K), AX.X, OP.add
            )
            ysq = wpool.tile([128, n_yt * K], f32, tag="xsq", name="ysq")
            nc.vector.tensor_tensor(ysq, ysb, ysb, OP.mult)
            nc.vector.tensor_reduce(
                y2t, ysq.rearrange("p (t k) -> p t k", k=K), AX.X, OP.add
            )

            # K-major fp16 operands via PE transpose (+ dtype cast on copy-out).
            # Split into part-tiles so the main loop's first matmuls only
            # depend on part 0 (whole-tile dep tracking otherwise serializes
            # all of phase 0 before the first matmul).
            n_xp = 2 if n_nt >= 2 else 1
            n_yp = n_jt
            XP = n_rows // n_xp
            YP = m_cols // n_yp
            xt_parts = [
                cpool.tile([KA, XP], f16, name=f"xtp{i}") for i in range(n_xp)
            ]
            yt_parts = [
                cpool.tile([KA, YP], f16, name=f"ytp{i}") for i in range(n_yp)
            ]

            ONE2 = 0x3C003C00  # two packed fp16 1.0s

            # y parts first: the first matmul needs y part 0 + x part 0.
            x2p = ps_pool.tile([128, JT], f32, tag="ps", name="x2p")
            nc.tensor.transpose(x2p[:n_nt, 0:128], x2t, ident32)
            x2r = wpool.tile([n_nt, 128], f16, tag="x2r", name="x2r")
            nc.scalar.copy(x2r, x2p[:n_nt, 0:128])
            y2p = ps_pool.tile([128, JT], f32, tag="ps", name="y2p")
            nc.tensor.transpose(y2p[:n_yt, 0:128], y2t, ident32)
            y2r = wpool.tile([n_yt, 128], f16, tag="x2r", name="y2r")
            nc.scalar.copy(y2r, y2p[:n_yt, 0:128])

            def build_y_part(i):
                yt = yt_parts[i]
                for t in range(i * (YP // 128), (i + 1) * (YP // 128)):
                    tp = ps_pool.tile([128, JT], f32, tag="ps", name="tp")
                    nc.tensor.transpose(
                        tp[:K, 0:128], ysb[:, t * K : (t + 1) * K], ident32
                    )
                    nc.scalar.copy(
                        yt[0:K, (t * 128) % YP : (t * 128) % YP + 128],
                        tp[:K, 0:128],
                    )
                nc.vector.memset(yt[K : K + 2, :].bitcast(u32), ONE2)
                nc.sync.dma_start(
                    yt[K + 1 : K + 2, :], y2r[i * (YP // 128) : (i + 1) * (YP // 128), :]
                )

            def build_x_part(i):
                xt = xt_parts[i]
                for t in range(i * (XP // 128), (i + 1) * (XP // 128)):
                    tp = ps_pool.tile([128, JT], f32, tag="ps", name="tp")
                    nc.tensor.transpose(
                        tp[:K, 0:128], xsb[:, t * K : (t + 1) * K], ident32
                    )
                    nc.scalar.mul(
                        xt[0:K, (t * 128) % XP : (t * 128) % XP + 128],
                        tp[:K, 0:128],
                        -2.0,
                    )
                nc.vector.memset(xt[K : K + 2, :].bitcast(u32), ONE2)
                nc.sync.dma_start(
                    xt[K : K + 1, :], x2r[i * (XP // 128) : (i + 1) * (XP // 128), :]
                )

            build_y_part(0)
            build_x_part(0)
            for i in range(1, n_yp):
                build_y_part(i)
            for i in range(1, n_xp):
                build_x_part(i)

            # ---------------- Phase 1: main flash loop ---------------------
            # t outer, m-superblocks inner; JT/MT matmuls fill each psum tile.
            rowmin2d = cpool.tile([128, n_nt], f32, name="rowmin2d")
            colmin2d = cpool.tile([128, m_cols // 128], f32, name="colmin2d")
            colacc = [
                cpool.tile([128, JT], f16, tag=f"colacc{j}", name=f"colacc{j}")
                for j in range(n_jt)
            ]

            for t in range(n_nt):
                xt = xt_parts[(t * 128) // XP]
                xo = (t * 128) % XP
                rowacc = wpool.tile([128, JT], f16, tag="rowacc", name="rowacc", bufs=4)
                for jj in range(n_jt):
                    yt = yt_parts[(jj * JT) // YP]
                    yo = (jj * JT) % YP
                    ps = ps_pool.tile([128, JT], f32, tag="ps", name="ps")
                    for h in range(JT // MT):
                        nc.tensor.matmul(
                            ps[:, h * MT : (h + 1) * MT],
                            lhsT=xt[:, xo : xo + 128],
                            rhs=yt[:, yo + h * MT : yo + (h + 1) * MT],
                            start=True,
                            stop=True,
                        )
                    if jj == 0:
                        # first superblock lands straight in rowacc
                        nc.scalar.copy(rowacc, ps)
                        src = rowacc
                    else:
                        tsb = tsb_pool.tile([128, JT], f16, tag="tsb", name="tsb", bufs=6)
                        nc.scalar.copy(tsb, ps)
                        nc.vector.tensor_tensor(rowacc, tsb, rowacc, OP.min)
                        src = tsb

                    if t == 0:
                        nc.vector.tensor_copy(colacc[jj], src)
                    else:
                        nc.vector.tensor_tensor(colacc[jj], src, colacc[jj], OP.min)

                # min over m for this n-tile (overlaps next t's matmuls):
                # fold halves twice with 2x TTs, then a 1x reduce on JT/4
                half = JT // 2
                nc.vector.tensor_tensor(
                    rowacc[:, 0:half], rowacc[:, 0:half], rowacc[:, half:JT], OP.min
                )
                quart = JT // 4
                nc.vector.tensor_tensor(
                    rowacc[:, 0:quart],
                    rowacc[:, 0:quart],
                    rowacc[:, quart : 2 * quart],
                    OP.min,
                )
                nc.vector.tensor_reduce(
                    rowmin2d[:, t : t + 1], rowacc[:, 0:quart], AX.X, OP.min
                )

            # partition-axis min of each colacc via PE transpose (tail)
            for jj in range(n_jt):
                for s in range(JT // 128):
                    cp = ps_pool.tile([128, JT], f16, tag="ps", name="cp")
                    nc.tensor.transpose(
                        cp[:, 0:128], colacc[jj][:, s * 128 : (s + 1) * 128], ident16
                    )
                    nc.vector.tensor_reduce(
                        colmin2d[:, jj * (JT // 128) + s : jj * (JT // 128) + s + 1],
                        cp[:, 0:128],
                        AX.X,
                        OP.min,
                    )

            # ---------------- Phase 2: writeback ---------------------------
            nc.sync.dma_start(outd[:, 0:n_nt], rowmin2d)
            nc.sync.dma_start(outd[:, n_nt:], colmin2d)

    nc.compile()
    return nc


def _get(n_rows, m_cols, num_cores):
    key = (n_rows, m_cols, num_cores)
    if key not in _COMPILED:
        _COMPILED[key] = _build(n_rows, m_cols, num_cores)
    return _COMPILED[key]


def _run(x, y, n_rows, m_cols, num_cores, trace=False):
    """x, y: [num_cores, n_rows|m_cols, K] fp32. Returns per-core out arrays."""
    global LAST_RESULTS
    from concourse import bass_utils

    nc = _get(n_rows, m_cols, num_cores)
    in_maps = [
        {"x": np.ascontiguousarray(x[b]), "y": np.ascontiguousarray(y[b])}
        for b in range(num_cores)
    ]
    res = bass_utils.run_bass_kernel_spmd(
        nc, in_maps, core_ids=list(range(num_cores)), trace=trace
    )
    LAST_RESULTS = res
    return [r["out"] for r in res.results]


def _postprocess(outs, n_rows, m_cols):
    """Host-side unshard: clamp, sqrt, mean. outs: list of [128, n_nt+m/128]."""
    n_nt = n_rows // NT
    total = 0.0
    for o in outs:
        rowmin = o[:, :n_nt].astype(np.float64)
        colmin = o[:, n_nt:].astype(np.float64)
        d1 = np.sqrt(np.maximum(rowmin, 0.0)).mean()  # min over m, mean over n
        d0 = np.sqrt(np.maximum(colmin, 0.0)).mean()  # min over n, mean over m
        total += d0 + d1
    return np.float32(total / len(outs))


def kernel(input1, input2):
    x = np.asarray(input1, dtype=np.float32)
    y = np.asarray(input2, dtype=np.float32)
    assert x.shape == (B, N, K) and y.shape == (B, M, K), (x.shape, y.shape)
    outs = _run(x, y, N, M, B)
    return _postprocess(outs, N, M)


# revision 17
# speedup vs baseline: 2.3709x; 1.0889x over previous
"""Chamfer distance kernel for Trainium2 (Bass/Tile), SPMD over 8 NeuronCores.

Problem: input1 [8, 4096, 64], input2 [8, 4096, 64] (fp32).
    D[b,n,m] = ||x_bn - y_bm||_2
    loss = mean_b( mean_m(min_n D) + mean_n(min_m D) )

Sharding: data-parallel over batch B=8 -> one batch element per core.

Per-core algorithm (flash-style, the [N, M] matrix never hits HBM):
  - Build augmented K-major fp16 operands so one matmul produces the full
    squared distance tile directly in PSUM (fp16 matmul streams at 1 cyc/col
    vs 4 for fp32; quantization impact on the final loss measured ~1e-6):
        lhsT = [ -2*X^T ; x2 ; 1 ]   (66 x 128 per n-tile)
        rhs  = [  Y^T   ; 1  ; y2 ]  (66 x 512 per m-tile)
        psum[n, m] = x2[n] + y2[m] - 2*<x_n, y_m>   = d^2
  - Four matmuls fill a 2048-wide 4-bank PSUM tile; ScalarE copies it to
    SBUF as fp16 (min-selection in fp16 is exact-to-selection). The first
    superblock copy lands directly in rowacc (saves a DVE copy).
  - VectorE: running fp16 min into rowacc (per n-tile, then fold+reduce to
    rowmin) and colacc[jj] (min over n-tiles) at the DVE 2x_1p rate.
  - colacc partition-axis min via TensorE transpose + free-axis reduce_min.
  - Device returns raw squared minima [128, n_nt + m/128]; host does
    clamp/sqrt/mean (cheap: 8192 values per core).
"""

import sys

if "/opt/trn_rl_repo" not in sys.path:
    sys.path.insert(0, "/opt/trn_rl_repo")

import numpy as np

B = 8
N = 4096
M = 4096
K = 64
NT = 128          # n-tile (psum partition dim)
MT = 512          # single-matmul moving free dim (one PSUM bank fp32)
KA = K + 2        # augmented contraction

_COMPILED = {}
LAST_RESULTS = None


def _build(n_rows, m_cols, num_cores):
    """Trace + compile the per-core bass program for [n_rows, K] x [m_cols, K]."""
    import concourse.bacc as bacc
    import concourse.mybir as mybir
    import concourse.tile as tile
    from concourse.masks import make_identity

    f32 = mybir.dt.float32
    f16 = mybir.dt.float16
    u32 = mybir.dt.uint32
    AX = mybir.AxisListType
    OP = mybir.AluOpType

    JT = min(2048, m_cols)      # m superblock (4 PSUM banks at 2048)
    n_nt = n_rows // NT
    n_jt = m_cols // JT
    n_yt = m_cols // 128        # y transpose tiles

    nc = bacc.Bacc(
        "TRN2", target_bir_lowering=False, debug=False, num_devices=num_cores
    )
    xd = nc.dram_tensor("x", [n_rows, K], f32, kind="ExternalInput")
    yd = nc.dram_tensor("y", [m_cols, K], f32, kind="ExternalInput")
    outd = nc.dram_tensor(
        "out", [128, n_nt + m_cols // 128], f32, kind="ExternalOutput"
    )

    with tile.TileContext(nc) as tc:
        with (
            tc.tile_pool(name="const", bufs=1) as cpool,
            tc.tile_pool(name="tsbp", bufs=4) as tsb_pool,
            tc.tile_pool(name="mpsum", bufs=2, space="PSUM") as ps_pool,
            tc.tile_pool(name="work", bufs=2) as wpool,
        ):
            # ---------------- Phase 0: load + build augmented operands -----
            xsb = cpool.tile([128, n_nt * K], f32, name="xsb")
            ysb = cpool.tile([128, n_yt * K], f32, name="ysb")
            nc.sync.dma_start(
                xsb.rearrange("p (t k) -> p t k", k=K),
                xd[:].rearrange("(t p) k -> p t k", p=128),
            )
            nc.sync.dma_start(
                ysb.rearrange("p (t k) -> p t k", k=K),
                yd[:].rearrange("(t p) k -> p t k", p=128),
            )

            ident32 = cpool.tile([128, 128], f32, name="ident32")
            make_identity(nc, ident32)
            ident16 = cpool.tile([128, 128], f16, name="ident16")
            make_identity(nc, ident16)

            # x2 / y2 per point: sum_k v^2, laid out [p, tile] (bulk DVE ops;
            # phase 0 is otherwise DVE-idle)
            x2t = cpool.tile([128, n_nt], f32, name="x2t")
            y2t = cpool.tile([128, n_yt], f32, name="y2t")
            xsq = wpool.tile([128, n_nt * K], f32, tag="xsq", name="xsq")
            nc.vector.tensor_tensor(xsq, xsb, xsb, OP.mult)
            nc.vector.tensor_reduce(
                x2t, xsq.rearrange("p (t k) -> p t k", k=K), AX.X, OP.add
            )
            ysq = wpool.tile([128, n_yt * K], f32, tag="xsq", name="ysq")
            nc.vector.tensor_tensor(ysq, ysb, ysb, OP.mult)
            nc.vector.tensor_reduce(
                y2t, ysq.rearrange("p (t k) -> p t k", k=K), AX.X, OP.add
            )

            # K-major fp16 operands via PE transpose (+ dtype cast on copy-out).
            # Split into part-tiles so the main loop's first matmuls only
            # depend on part 0 (whole-tile dep tracking otherwise serializes
            # all of phase 0 before the first matmul).
            n_xp = 2 if n_nt >= 2 else 1
            n_yp = n_jt
            XP = n_rows // n_xp
            YP = m_cols // n_yp
            xt_parts = [
                cpool.tile([KA, XP], f16, name=f"xtp{i}") for i in range(n_xp)
            ]
            yt_parts = [
                cpool.tile([KA, YP], f16, name=f"ytp{i}") for i in range(n_yp)
            ]

            ONE2 = 0x3C003C00  # two packed fp16 1.0s

            # y parts first: the first matmul needs y part 0 + x part 0.
            x2p = ps_pool.tile([128, JT], f32, tag="ps", name="x2p")
            nc.tensor.transpose(x2p[:n_nt, 0:128], x2t, ident32)
            x2r = wpool.tile([n_nt, 128], f16, tag="x2r", name="x2r")
            nc.scalar.copy(x2r, x2p[:n_nt, 0:128])
            y2p = ps_pool.tile([128, JT], f32, tag="ps", name="y2p")
            nc.tensor.transpose(y2p[:n_yt, 0:128], y2t, ident32)
            y2r = wpool.tile([n_yt, 128], f16, tag="x2r", name="y2r")
            nc.scalar.copy(y2r, y2p[:n_yt, 0:128])

            # Batched transposes: up to 16 [64,128] transpose results land
            # side-by-side in one psum tile, drained by ONE wide ACT copy.
            def build_y_part(i):
                yt = yt_parts[i]
                t0 = i * (YP // 128)
                for c0 in range(0, YP, JT):
                    w = min(JT, YP - c0)
                    tp = ps_pool.tile([128, JT], f32, tag="ps", name="tp")
                    for j in range(w // 128):
                        t = t0 + (c0 + j * 128) // 128
                        nc.tensor.transpose(
                            tp[:K, j * 128 : (j + 1) * 128],
                            ysb[:, t * K : (t + 1) * K],
                            ident32,
                        )
                    nc.scalar.copy(yt[0:K, c0 : c0 + w], tp[:K, 0:w])
                nc.vector.memset(yt[K : K + 2, :].bitcast(u32), ONE2)
                nc.sync.dma_start(
                    yt[K + 1 : K + 2, :], y2r[i * (YP // 128) : (i + 1) * (YP // 128), :]
                )

            def build_x_part(i):
                xt = xt_parts[i]
                t0 = i * (XP // 128)
                for c0 in range(0, XP, JT):
                    w = min(JT, XP - c0)
                    tp = ps_pool.tile([128, JT], f32, tag="ps", name="tp")
                    for j in range(w // 128):
                        t = t0 + (c0 + j * 128) // 128
                        nc.tensor.transpose(
                            tp[:K, j * 128 : (j + 1) * 128],
                            xsb[:, t * K : (t + 1) * K],
                            ident32,
                        )
                    nc.scalar.mul(xt[0:K, c0 : c0 + w], tp[:K, 0:w], -2.0)
                nc.vector.memset(xt[K : K + 2, :].bitcast(u32), ONE2)
                nc.sync.dma_start(
                    xt[K : K + 1, :], x2r[i * (XP // 128) : (i + 1) * (XP // 128), :]
                )

            build_y_part(0)
            build_x_part(0)

            # ---------------- Phase 1: main flash loop ---------------------
            # t outer, m-superblocks inner; JT/MT matmuls fill each psum tile.
            rowmin2d = cpool.tile([128, n_nt], f32, name="rowmin2d")
            colmin2d = cpool.tile([128, m_cols // 128], f32, name="colmin2d")
            colacc = [
                cpool.tile([128, JT], f16, tag=f"colacc{j}", name=f"colacc{j}")
                for j in range(n_jt)
            ]

            for t in range(n_nt):
                # interleave remaining operand builds into the first iterations
                if t == 0:
                    for i in range(1, n_yp):
                        build_y_part(i)
                elif 1 <= t < n_xp:
                    build_x_part(t)
                xt = xt_parts[(t * 128) // XP]
                xo = (t * 128) % XP
                rowacc = wpool.tile([128, JT], f16, tag="rowacc", name="rowacc", bufs=4)
                for jj in range(n_jt):
                    yt = yt_parts[(jj * JT) // YP]
                    yo = (jj * JT) % YP
                    ps = ps_pool.tile([128, JT], f32, tag="ps", name="ps")
                    for h in range(JT // MT):
                        nc.tensor.matmul(
                            ps[:, h * MT : (h + 1) * MT],
                            lhsT=xt[:, xo : xo + 128],
                            rhs=yt[:, yo + h * MT : yo + (h + 1) * MT],
                            start=True,
                            stop=True,
                        )
                    if jj == 0:
                        # first superblock lands straight in rowacc
                        nc.scalar.copy(rowacc, ps)
                        src = rowacc
                    else:
                        tsb = tsb_pool.tile([128, JT], f16, tag="tsb", name="tsb", bufs=6)
                        nc.scalar.copy(tsb, ps)
                        nc.vector.tensor_tensor(rowacc, tsb, rowacc, OP.min)
                        src = tsb

                    if t == 0:
                        nc.vector.tensor_copy(colacc[jj], src)
                    else:
                        nc.vector.tensor_tensor(colacc[jj], src, colacc[jj], OP.min)

                # min over m for this n-tile (overlaps next t's matmuls):
                # fold halves twice with 2x TTs, then a 1x reduce on JT/4
                half = JT // 2
                nc.vector.tensor_tensor(
                    rowacc[:, 0:half], rowacc[:, 0:half], rowacc[:, half:JT], OP.min
                )
                quart = JT // 4
                nc.vector.tensor_tensor(
                    rowacc[:, 0:quart],
                    rowacc[:, 0:quart],
                    rowacc[:, quart : 2 * quart],
                    OP.min,
                )
                nc.vector.tensor_reduce(
                    rowmin2d[:, t : t + 1], rowacc[:, 0:quart], AX.X, OP.min
                )

            # partition-axis min of each colacc via PE transpose (tail)
            for jj in range(n_jt):
                for s in range(JT // 128):
                    cp = ps_pool.tile([128, JT], f16, tag="ps", name="cp")
                    nc.tensor.transpose(
                        cp[:, 0:128], colacc[jj][:, s * 128 : (s + 1) * 128], ident16
                    )
                    nc.vector.tensor_reduce(
                        colmin2d[:, jj * (JT // 128) + s : jj * (JT // 128) + s + 1],
                        cp[:, 0:128],
                        AX.X,
                        OP.min,
                    )

            # ---------------- Phase 2: writeback ---------------------------
            nc.sync.dma_start(outd[:, 0:n_nt], rowmin2d)
            nc.sync.dma_start(outd[:, n_nt:], colmin2d)

    nc.compile()
    return nc


def _get(n_rows, m_cols, num_cores):
    key = (n_rows, m_cols, num_cores)
    if key not in _COMPILED:
        _COMPILED[key] = _build(n_rows, m_cols, num_cores)
    return _COMPILED[key]


def _run(x, y, n_rows, m_cols, num_cores, trace=False):
    """x, y: [num_cores, n_rows|m_cols, K] fp32. Returns per-core out arrays."""
    global LAST_RESULTS
    from concourse import bass_utils

    nc = _get(n_rows, m_cols, num_cores)
    in_maps = [
        {"x": np.ascontiguousarray(x[b]), "y": np.ascontiguousarray(y[b])}
        for b in range(num_cores)
    ]
    res = bass_utils.run_bass_kernel_spmd(
        nc, in_maps, core_ids=list(range(num_cores)), trace=trace
    )
    LAST_RESULTS = res
    return [r["out"] for r in res.results]


def _postprocess(outs, n_rows, m_cols):
    """Host-side unshard: clamp, sqrt, mean. outs: list of [128, n_nt+m/128]."""
    n_nt = n_rows // NT
    total = 0.0
    for o in outs:
        rowmin = o[:, :n_nt].astype(np.float64)
        colmin = o[:, n_nt:].astype(np.float64)
        d1 = np.sqrt(np.maximum(rowmin, 0.0)).mean()  # min over m, mean over n
        d0 = np.sqrt(np.maximum(colmin, 0.0)).mean()  # min over n, mean over m
        total += d0 + d1
    return np.float32(total / len(outs))


def kernel(input1, input2):
    x = np.asarray(input1, dtype=np.float32)
    y = np.asarray(input2, dtype=np.float32)
    assert x.shape == (B, N, K) and y.shape == (B, M, K), (x.shape, y.shape)
    outs = _run(x, y, N, M, B)
    return _postprocess(outs, N, M)


# revision 33
# speedup vs baseline: 2.7007x; 1.1391x over previous
"""Chamfer distance kernel for Trainium2 (Bass/Tile), SPMD over 8 NeuronCores.

Problem: input1 [8, 4096, 64], input2 [8, 4096, 64] (fp32).
    D[b,n,m] = ||x_bn - y_bm||_2
    loss = mean_b( mean_m(min_n D) + mean_n(min_m D) )

Sharding: data-parallel over batch B=8 -> one batch element per core.

Per-core algorithm (flash-style, the [N, M] matrix never hits HBM):
  - Build augmented K-major fp16 operands so one matmul produces the full
    squared distance tile directly in PSUM (fp16 matmul streams at 1 cyc/col
    vs 4 for fp32; quantization impact on the final loss measured ~1e-6):
        lhsT = [ -2*X^T ; 1 ]   (65 x 128 per n-tile)
        rhs  = [  Y^T  ; y2 ]   (65 x 512 per m-tile)
        psum[n, m] = y2[m] - 2*<x_n, y_m>;  x2[n] is added for free as the
        per-partition bias of the ScalarE psum->SBUF copy  -> d^2
  - Four matmuls fill a 2048-wide 4-bank PSUM tile; ScalarE copies it to
    SBUF as fp16 (min-selection in fp16 is exact-to-selection). The first
    superblock copy lands directly in rowacc (saves a DVE copy).
  - VectorE: running fp16 min into rowacc (per n-tile, then fold+reduce to
    rowmin) and colacc[jj] (min over n-tiles) at the DVE 2x_1p rate.
  - Device returns rowmin [128, n_nt] f32 plus the colacc planes [128, M]
    f16; host finishes with the partition-axis column min + clamp/sqrt/mean
    (a few thousand values per core).

Measured on the 8-core axon TRN2 pod: HW exec ~194 us, loss rel err ~1.2e-7.
"""

import sys

if "/opt/trn_rl_repo" not in sys.path:
    sys.path.insert(0, "/opt/trn_rl_repo")

import numpy as np

B = 8
N = 4096
M = 4096
K = 64
NT = 128          # n-tile (psum partition dim)
MT = 512          # single-matmul moving free dim (one PSUM bank fp32)
KA = K + 1        # augmented contraction (ones row / y2 row)

_COMPILED = {}
LAST_RESULTS = None


def _build(n_rows, m_cols, num_cores):
    """Trace + compile the per-core bass program for [n_rows, K] x [m_cols, K]."""
    import concourse.bacc as bacc
    import concourse.mybir as mybir
    import concourse.tile as tile
    from concourse.masks import make_identity

    f32 = mybir.dt.float32
    f16 = mybir.dt.float16
    u32 = mybir.dt.uint32
    AX = mybir.AxisListType
    OP = mybir.AluOpType

    JT = min(2048, m_cols)      # m superblock (4 PSUM banks at 2048)
    n_nt = n_rows // NT
    n_jt = m_cols // JT
    n_yt = m_cols // 128        # y transpose tiles

    nc = bacc.Bacc(
        "TRN2", target_bir_lowering=False, debug=False, num_devices=num_cores
    )
    xd = nc.dram_tensor("x", [n_rows, K], f32, kind="ExternalInput")
    yd = nc.dram_tensor("y", [m_cols, K], f32, kind="ExternalInput")
    outd = nc.dram_tensor("out", [128, n_nt], f32, kind="ExternalOutput")
    outc = nc.dram_tensor("outc", [128, m_cols], f16, kind="ExternalOutput")

    with tile.TileContext(nc) as tc:
        with (
            tc.tile_pool(name="const", bufs=1) as cpool,
            tc.tile_pool(name="tsbp", bufs=4) as tsb_pool,
            tc.tile_pool(name="mpsum", bufs=2, space="PSUM") as ps_pool,
            tc.tile_pool(name="work", bufs=2) as wpool,
        ):
            # ---------------- Phase 0: load + build augmented operands -----
            # y side first everywhere: the first matmul's longest dependency
            # chain is ysb -> y2 -> y2-row DMA -> yt part 0.
            xsb = cpool.tile([128, n_nt * K], f32, name="xsb")
            ysb = cpool.tile([128, n_yt * K], f32, name="ysb")
            # partition-major load: each partition gets a contiguous 8KB run
            # of DRAM rows (128 big DMA descriptors instead of 4096 small).
            # This permutes the n/m identity of every tile column, which is
            # harmless: both outputs are reduced by means on the host.
            nc.sync.dma_start(ysb, yd[:].rearrange("(p r) k -> p (r k)", p=128))
            nc.sync.dma_start(xsb, xd[:].rearrange("(p r) k -> p (r k)", p=128))

            ident32 = cpool.tile([128, 128], f32, name="ident32")
            make_identity(nc, ident32)

            # x2 / y2 per point: sum_k v^2, laid out [p, tile] (bulk DVE ops;
            # phase 0 is otherwise DVE-idle)
            x2t = cpool.tile([128, n_nt], f32, name="x2t")
            y2t = cpool.tile([128, n_yt], f32, name="y2t")
            ysq = wpool.tile([128, n_yt * K], f32, tag="xsq", name="ysq")
            nc.vector.tensor_tensor(ysq, ysb, ysb, OP.mult)
            nc.vector.tensor_reduce(
                y2t, ysq.rearrange("p (t k) -> p t k", k=K), AX.X, OP.add
            )
            xsq = wpool.tile([128, n_nt * K], f32, tag="xsq", name="xsq")
            nc.vector.tensor_tensor(xsq, xsb, xsb, OP.mult)
            nc.vector.tensor_reduce(
                x2t, xsq.rearrange("p (t k) -> p t k", k=K), AX.X, OP.add
            )

            # K-major fp16 operands via PE transpose (+ dtype cast on copy-out).
            # Split into part-tiles so the main loop's first matmuls only
            # depend on part 0 (whole-tile dep tracking otherwise serializes
            # all of phase 0 before the first matmul).
            n_xp = 2 if n_nt >= 2 else 1
            n_yp = n_jt
            XP = n_rows // n_xp
            YP = m_cols // n_yp
            xt_parts = [
                cpool.tile([KA, XP], f16, name=f"xtp{i}") for i in range(n_xp)
            ]
            yt_parts = [
                cpool.tile([KA, YP], f16, name=f"ytp{i}") for i in range(n_yp)
            ]

            ONE2 = 0x3C003C00  # two packed fp16 1.0s

            # y parts first: the first matmul needs y part 0 + x part 0.
            y2p = ps_pool.tile([128, JT], f32, tag="ps", name="y2p")
            nc.tensor.transpose(y2p[:n_yt, 0:128], y2t, ident32)
            y2r = wpool.tile([n_yt, 128], f16, tag="x2r", name="y2r")
            nc.scalar.copy(y2r, y2p[:n_yt, 0:128])

            # Batched transposes: up to 16 [64,128] transpose results land
            # side-by-side in one psum tile, drained by ONE wide ACT copy.
            def build_y_part(i):
                yt = yt_parts[i]
                t0 = i * (YP // 128)
                for c0 in range(0, YP, JT):
                    w = min(JT, YP - c0)
                    tp = ps_pool.tile([128, JT], f32, tag="ps", name="tp")
                    for j in range(w // 128):
                        t = t0 + (c0 + j * 128) // 128
                        nc.tensor.transpose(
                            tp[:K, j * 128 : (j + 1) * 128],
                            ysb[:, t * K : (t + 1) * K],
                            ident32,
                        )
                    nc.scalar.copy(yt[0:K, c0 : c0 + w], tp[:K, 0:w])
                nc.sync.dma_start(
                    yt[K : K + 1, :], y2r[i * (YP // 128) : (i + 1) * (YP // 128), :]
                )

            def build_x_part(i):
                xt = xt_parts[i]
                t0 = i * (XP // 128)
                for c0 in range(0, XP, JT):
                    w = min(JT, XP - c0)
                    tp = ps_pool.tile([128, JT], f32, tag="ps", name="tp")
                    for j in range(w // 128):
                        t = t0 + (c0 + j * 128) // 128
                        nc.tensor.transpose(
                            tp[:K, j * 128 : (j + 1) * 128],
                            xsb[:, t * K : (t + 1) * K],
                            ident32,
                        )
                    nc.scalar.mul(xt[0:K, c0 : c0 + w], tp[:K, 0:w], -2.0)
                nc.gpsimd.memset(xt[K : K + 1, :].bitcast(u32), ONE2)

            build_y_part(0)
            build_x_part(0)

            # ---------------- Phase 1: main flash loop ---------------------
            # t outer, m-superblocks inner; JT/MT matmuls fill each psum tile.
            rowmin2d = cpool.tile([128, n_nt], f32, name="rowmin2d")
            colmin2d = cpool.tile([128, m_cols // 128], f32, name="colmin2d")
            colacc = [
                cpool.tile([128, JT], f16, tag=f"colacc{j}", name=f"colacc{j}")
                for j in range(n_jt)
            ]

            for t in range(n_nt):
                # interleave remaining x-part builds a few iterations in
                # (x part i is not needed until t = i * XP/128)
                if t == max(1, min(4, XP // 128 - 1)):
                    for i in range(1, n_xp):
                        build_x_part(i)
                xt = xt_parts[(t * 128) // XP]
                xo = (t * 128) % XP
                rowacc = wpool.tile([128, JT], f16, tag="rowacc", name="rowacc", bufs=6)
                for jj in range(n_jt):
                    # y part jj is first read here; build it just in time so
                    # it does not delay earlier matmuls in the PE stream
                    if t == 0 and jj >= 1:
                        build_y_part(jj)
                    yt = yt_parts[(jj * JT) // YP]
                    yo = (jj * JT) % YP
                    ps = ps_pool.tile([128, JT], f32, tag="ps", name="ps")
                    for h in range(JT // MT):
                        nc.tensor.matmul(
                            ps[:, h * MT : (h + 1) * MT],
                            lhsT=xt[:, xo : xo + 128],
                            rhs=yt[:, yo + h * MT : yo + (h + 1) * MT],
                            start=True,
                            stop=True,
                        )
                    x2col = x2t[:, t : t + 1]
                    if jj == 0:
                        # first superblock lands straight in rowacc; the
                        # per-partition bias adds x2[n] for free on ScalarE
                        nc.scalar.add(rowacc, ps, x2col)
                        src = rowacc
                    else:
                        tsb = tsb_pool.tile([128, JT], f16, tag="tsb", name="tsb", bufs=8)
                        nc.scalar.add(tsb, ps, x2col)
                        nc.vector.tensor_tensor(rowacc, tsb, rowacc, OP.min)
                        src = tsb

                    if t == 0:
                        nc.vector.tensor_copy(colacc[jj], src)
                    else:
                        nc.vector.tensor_tensor(colacc[jj], src, colacc[jj], OP.min)

                # min over m for this n-tile (overlaps next t's matmuls):
                # fold halves twice with 2x TTs, then a 1x reduce on JT/4
                half = JT // 2
                nc.vector.tensor_tensor(
                    rowacc[:, 0:half], rowacc[:, 0:half], rowacc[:, half:JT], OP.min
                )
                quart = JT // 4
                nc.vector.tensor_tensor(
                    rowacc[:, 0:quart],
                    rowacc[:, 0:quart],
                    rowacc[:, quart : 2 * quart],
                    OP.min,
                )
                eighth = JT // 8
                nc.vector.tensor_tensor(
                    rowacc[:, 0:eighth],
                    rowacc[:, 0:eighth],
                    rowacc[:, eighth : 2 * eighth],
                    OP.min,
                )
                nc.vector.tensor_reduce(
                    rowmin2d[:, t : t + 1], rowacc[:, 0:eighth], AX.X, OP.min
                )

            # ---------------- Phase 2: writeback ---------------------------
            # colacc partition-axis min happens on the host (4096 cols/core)
            for jj in range(n_jt):
                nc.sync.dma_start(outc[:, jj * JT : (jj + 1) * JT], colacc[jj])
            nc.sync.dma_start(outd[:, 0:n_nt], rowmin2d)

    nc.compile()
    return nc


def _get(n_rows, m_cols, num_cores):
    key = (n_rows, m_cols, num_cores)
    if key not in _COMPILED:
        _COMPILED[key] = _build(n_rows, m_cols, num_cores)
    return _COMPILED[key]


def _run(x, y, n_rows, m_cols, num_cores, trace=False):
    """x, y: [num_cores, n_rows|m_cols, K] fp32. Returns per-core out arrays."""
    global LAST_RESULTS
    from concourse import bass_utils

    nc = _get(n_rows, m_cols, num_cores)
    in_maps = [
        {"x": np.ascontiguousarray(x[b]), "y": np.ascontiguousarray(y[b])}
        for b in range(num_cores)
    ]
    res = bass_utils.run_bass_kernel_spmd(
        nc, in_maps, core_ids=list(range(num_cores)), trace=trace
    )
    LAST_RESULTS = res
    return [r["out"] for r in res.results]


def _postprocess(outs, n_rows, m_cols):
    """Host-side unshard: clamp, sqrt, mean. outs: list of [128, n_nt+m/128]."""
    n_nt = n_rows // NT
    total = 0.0
    for o in outs:
        rowmin = o[:, :n_nt].astype(np.float64)
        colmin = o[:, n_nt:].astype(np.float64)
        d1 = np.sqrt(np.maximum(rowmin, 0.0)).mean()  # min over m, mean over n
        d0 = np.sqrt(np.maximum(colmin, 0.0)).mean()  # min over n, mean over m
        total += d0 + d1
    return np.float32(total / len(outs))


def kernel(input1, input2):
    x = np.asarray(input1, dtype=np.float32)
    y = np.asarray(input2, dtype=np.float32)
    assert x.shape == (B, N, K) and y.shape == (B, M, K), (x.shape, y.shape)
    outs = _run(x, y, N, M, B)
    return _postprocess(outs, N, M)
